# revision 22
# baseline (speedup 1.0000x reference)
"""Trainium2 Bass kernel for the slimmable-conv MoE-routing module.

Reference computation (B=16, C=128, L=32768, G=4):
  pool   = mean(x, axis=-1)                      [B, C]
  logits = pool @ w_gate.T                       [B, G]
  gate   = straight-through gumbel softmax       [B, G]  (~one-hot)
  z      = conv_w @ x + conv_b                   [B, C, L]  (pointwise conv)
  out1   = z * (gate @ MASK)                     (channel gating)
  xn     = (out1 - gate@rmean) / sqrt(gate@rvar + eps) * bn_w + bn_b
  out    = xn * (gate @ MASK)

Everything after the pool reduces to a per-(batch,channel) affine applied to
the conv output:  out[b,c,l] = z_mm[b,c,l] * S[b,c] + T[b,c]  where z_mm is
the pure matmul part and S/T fold the gate, conv bias and BN constants.
The per-gate-choice tables S_all/T_all [G, C], the transposed gate weight
and the bf16 transposed conv weight are all tiny and input-independent of x,
so they are precomputed on the host in kernel() and passed as inputs.

Sharding: data-parallel over batch, 2 batches per core, 8 cores.  HBM
traffic per core is the 64 MiB floor (read x once, write out once): ALL of
z stays resident in SBUF as bf16 (0.2% rounding ~ well under the 2e-2
tolerance).  Peak residency is one batch + lookahead, not two: batch 1's z
chunks are written into the slots that batch 0's epilogue frees as its
outputs stream out.

Per-core schedule (16 chunks of 2048 cols per batch):
  phase A0 : stream b0 chunks: DMA in (Sync ring; consts ride the ACT
             ring), one DVE op casts to bf16 AND accumulates the pool
             partial-sum, PE bf16 z=conv_w@x, ACT drains PSUM -> bf16 slot.
  prefetch : b1 chunks 0..SPARE-1 -> spare slots (keeps DMA busy while
             the gate chain runs).
  gate(0)  : pool -> logits(+gumbel via PSUM accum) -> hard one-hot ->
             select S/T column via tiny matmuls.
  steady   : epilogue b0 chunk j (z*S+T -> fp32 staging -> DMA out,
             DVE/ACT alternating) interleaved with phase-A work for b1
             chunk j+SPARE into the just-freed slot.
  drain b0, gate(1), epilogue all b1 chunks (write-only tail).
"""

import ml_dtypes
import numpy as np

import concourse.bass as bass
import concourse.tile as tile
from concourse import mybir, bacc
from concourse.bass_utils import run_bass_kernel_spmd

F32 = mybir.dt.float32
BF16 = mybir.dt.bfloat16

B, C, L, G = 16, 128, 32768, 4
NCORES = 8
BPC = B // NCORES          # batches per core
CHANNELS = [32, 64, 96, 128]
EPS = 1e-5

LC = 2048                  # columns per DMA chunk
NCHUNK = L // LC           # chunks per batch
MMN = 512                  # matmul moving-dim
NMM = LC // MMN            # matmuls per chunk
SPARE = 9                  # extra z slots (b1 lookahead past gate(0))

AX = mybir.AxisListType.X
ALU = mybir.AluOpType
ACTF = mybir.ActivationFunctionType


def host_transform(w_gate, conv_w, conv_b, bn_w, bn_b, rmean, rvar,
                   l_total=L):
    """Input-side constant folding (exact fp32, matches the on-chip algebra
    that was validated against the reference).  wgT absorbs the pool's 1/L
    (L is a power of two, so the fold is exact)."""
    f = np.float32
    mask = (np.arange(C)[None, :] < np.asarray(CHANNELS)[:, None]).astype(f)
    istd = (f(1.0) / np.sqrt(np.asarray(rvar, f) + f(EPS))).astype(f)
    bw = np.asarray(bn_w, f).reshape(1, C)
    bb = np.asarray(bn_b, f).reshape(1, C)
    cb = np.asarray(conv_b, f).reshape(1, C)
    S = (mask * istd * bw).astype(f)                               # [G, C]
    T = (((cb * mask - np.asarray(rmean, f)) * istd * bw + bb) * mask).astype(f)
    return {
        "wgT": np.ascontiguousarray(
            np.asarray(w_gate, f).T * f(1.0 / l_total)),           # [C, G]
        "cwT": np.ascontiguousarray(
            np.asarray(conv_w, f).T).astype(ml_dtypes.bfloat16),   # [C, C]
        "sall": np.ascontiguousarray(S),
        "tall": np.ascontiguousarray(T),
    }


def build_kernel(l_total=L, n_res=None):
    nchunk = l_total // LC
    spare = min(SPARE, nchunk)
    nslot = nchunk + spare
    nc = bacc.Bacc("TRN2", target_bir_lowering=False)

    x_d = nc.declare_dram_parameter("x", [BPC, C, l_total], F32, isOutput=False)
    gum_d = nc.declare_dram_parameter("gumbel", [BPC, G], F32, isOutput=False)
    wg_d = nc.declare_dram_parameter("wgT", [C, G], F32, isOutput=False)
    cw_d = nc.declare_dram_parameter("cwT", [C, C], BF16, isOutput=False)
    sa_d = nc.declare_dram_parameter("sall", [G, C], F32, isOutput=False)
    ta_d = nc.declare_dram_parameter("tall", [G, C], F32, isOutput=False)
    out_d = nc.declare_dram_parameter("out", [BPC, C, l_total], F32, isOutput=True)

    def slot(b, ci):
        return ci if b == 0 else (nchunk + ci) % nslot

    with tile.TileContext(nc) as tc:
        with (
            tc.tile_pool(name="consts", bufs=1) as consts,
            tc.tile_pool(name="xin", bufs=6) as xin_pool,
            tc.tile_pool(name="xbf", bufs=3) as xbf_pool,
            tc.tile_pool(name="zres", bufs=1) as zres_pool,
            tc.tile_pool(name="stage", bufs=5) as stage_pool,
            tc.tile_pool(name="small", bufs=1) as small,
            tc.tile_pool(name="psz", bufs=5, space="PSUM") as psz,
            tc.tile_pool(name="pss", bufs=1, space="PSUM") as pss,
        ):
            # ---- prefetch the first x chunks; all consts DMAs ride the
            # ACT HWDGE ring so the Sync ring is a pure x/out stream ----
            npre = min(6, nchunk)
            pre = []
            for ci in range(npre):
                xc = xin_pool.tile([C, LC], F32, tag="xin", name=f"xpre{ci}")
                nc.sync.dma_start(
                    out=xc, in_=x_d.ap()[0, :, ci * LC:(ci + 1) * LC])
                pre.append(xc)

            # ---- constants (DMA-only; no on-chip compute) ----
            convwT = consts.tile([C, C], BF16)      # [i, o] = conv_w[o, i]
            nc.scalar.dma_start(out=convwT, in_=cw_d.ap())
            wgT = consts.tile([C, G], F32)          # [c, g] = w_gate[g, c]
            nc.scalar.dma_start(out=wgT, in_=wg_d.ap())
            S_allT = consts.tile([G, C], F32)
            nc.scalar.dma_start(out=S_allT, in_=sa_d.ap())
            T_allT = consts.tile([G, C], F32)
            nc.scalar.dma_start(out=T_allT, in_=ta_d.ap())
            gum_rows = []
            for b in range(BPC):
                gr = consts.tile([1, G], F32, tag=f"gum{b}")
                nc.scalar.dma_start(out=gr, in_=gum_d.ap()[b:b + 1, :])
                gum_rows.append(gr)

            one_sb = consts.tile([1, 1], F32)
            nc.vector.memset(one_sb, 1.0)
            partials = consts.tile([C, BPC * nchunk], F32)
            nc.vector.memset(partials, 0.0)
            pool_sb = consts.tile([C, BPC], F32)
            ST_sb = consts.tile([C, 2 * BPC], F32)

            zres = [zres_pool.tile([C, LC], BF16, tag=f"z{s}", name=f"zres{s}")
                    for s in range(nslot)]

            def phase_a_chunk(b, ci, xc=None, copy_eng="act"):
                col = b * nchunk + ci
                if xc is None:
                    xc = xin_pool.tile([C, LC], F32, tag="xin")
                    nc.sync.dma_start(
                        out=xc, in_=x_d.ap()[b, :, ci * LC:(ci + 1) * LC])
                # one DVE op: bf16 cast for the matmul + fp32 column-sum
                # (pool noise ~1e-5 logit shift vs 0.04 min gate gap)
                xbf = xbf_pool.tile([C, LC], BF16, tag="xbf")
                nc.vector.tensor_scalar(
                    out=xbf, in0=xc, scalar1=1.0, scalar2=None, op0=ALU.mult,
                    op1=ALU.add, accum_out=partials[:, col:col + 1])
                dst = zres[slot(b, ci)]
                for j in range(NMM):
                    js = slice(j * MMN, (j + 1) * MMN)
                    zp = psz.tile([C, MMN], F32)
                    nc.tensor.matmul(out=zp, lhsT=convwT, rhs=xbf[:, js],
                                     start=True, stop=True)
                    # during steady the PSUM drain must stay off ACT: ACT's
                    # FIFO is write-paced (affines wait on stage bufs) and
                    # would back-pressure PE -> xbf -> accums -> gate(1)
                    if copy_eng == "act":
                        nc.scalar.copy(out=dst[:, js], in_=zp)
                    else:
                        nc.vector.tensor_copy(out=dst[:, js], in_=zp)

            def finish_pool(b):
                # pool_sb holds column SUMS; the 1/L lives in wgT (host)
                nc.vector.reduce_sum(
                    out=pool_sb[:, b:b + 1],
                    in_=partials[:, b * nchunk:(b + 1) * nchunk],
                    axis=AX)

            def gate_phase(b):
                """Short gating chain: logits -> hard one-hot -> select
                precomputed S/T columns via tiny matmuls.  Kept as few
                serial cross-engine hops as possible (each costs ~1.5us)."""
                # y = pool @ wgT + 1*gumbel, accumulated in PSUM (2 PE ops)
                lg_ps = pss.tile([1, G], F32, tag="lg")
                nc.tensor.matmul(out=lg_ps, lhsT=pool_sb[:, b:b + 1], rhs=wgT,
                                 start=True, stop=False)
                nc.tensor.matmul(out=lg_ps, lhsT=one_sb, rhs=gum_rows[b],
                                 start=False, stop=True)
                m1 = small.tile([1, 1], F32, tag=f"m1{b}")
                nc.vector.reduce_max(out=m1, in_=lg_ps, axis=AX)
                yhard = small.tile([1, G], F32, tag=f"yh{b}")
                nc.vector.tensor_scalar(out=yhard, in0=lg_ps, scalar1=m1,
                                        scalar2=None, op0=ALU.is_ge)
                gt_ps = pss.tile([G, 1], F32, tag="gt")
                nc.tensor.transpose(out=gt_ps, in_=yhard, identity=one_sb)
                gateT = small.tile([G, 1], F32, tag=f"gT{b}")
                nc.vector.tensor_copy(out=gateT, in_=gt_ps)

                sel_ps = pss.tile([C, 2], F32, tag="big")
                nc.tensor.matmul(out=sel_ps[:, 0:1], lhsT=S_allT, rhs=gateT,
                                 start=True, stop=True)
                nc.tensor.matmul(out=sel_ps[:, 1:2], lhsT=T_allT, rhs=gateT,
                                 start=True, stop=True)
                nc.vector.tensor_copy(out=ST_sb[:, 2 * b:2 * b + 2],
                                      in_=sel_ps)

            epi_count = [0]

            def epilogue(b, ci, eng=None):
                """out[:, chunk] = zres * S + T   (bf16 -> fp32 staging)"""
                S_col = ST_sb[:, 2 * b:2 * b + 1]
                T_col = ST_sb[:, 2 * b + 1:2 * b + 2]
                zt = zres[slot(b, ci)]
                st = stage_pool.tile([C, LC], F32, tag="stage")
                use_vec = (epi_count[0] % 2 == 0) if eng is None else (eng == "vec")
                if use_vec:
                    nc.vector.tensor_scalar(
                        out=st, in0=zt, scalar1=S_col, scalar2=T_col,
                        op0=ALU.mult, op1=ALU.add)
                else:
                    nc.scalar.activation(out=st, in_=zt, func=ACTF.Identity,
                                         bias=T_col, scale=S_col)
                epi_count[0] += 1
                nc.sync.dma_start(
                    out=out_d.ap()[b, :, ci * LC:(ci + 1) * LC], in_=st)

            # ---- emission order ----
            # Mixed read+write DMA sustains ~400 GB/s vs ~330 write-only,
            # so steady interleaves TWO b0 epilogues per b1 fill: all of
            # b0's writes overlap b1's remaining reads.  gate(1) is emitted
            # straight after the last fill so its chain isn't queued behind
            # write-paced epilogues; the pure-write tail is then just b1.
            with nc.named_scope("phaseA0"):
                for ci in range(nchunk):
                    phase_a_chunk(0, ci, xc=pre[ci] if ci < npre else None)
            with nc.named_scope("prefetchB1"):
                for ci in range(spare):
                    phase_a_chunk(1, ci)
            with nc.named_scope("gate0"):
                finish_pool(0)
                gate_phase(0)
            epi_next = 0
            with nc.named_scope("steady"):
                for k, ci in enumerate(range(spare, nchunk)):
                    target = min(nchunk, max(ci - spare + 1, 2 * (k + 1)))
                    while epi_next < target:
                        epilogue(0, epi_next, eng="act")
                        epi_next += 1
                    phase_a_chunk(1, ci, copy_eng="vec")
            with nc.named_scope("gate1"):
                finish_pool(1)
                gate_phase(1)
            with nc.named_scope("drain"):
                while epi_next < nchunk:
                    epilogue(0, epi_next)
                    epi_next += 1
                for ci in range(nchunk):
                    epilogue(1, ci)

    nc.compile()
    return nc


_NC = None


def _get_nc():
    global _NC
    if _NC is None:
        _NC = build_kernel()
    return _NC


def kernel(x, gumbel_noise, w_gate, conv_w, conv_b, bn_w, bn_b, rmean, rvar):
    nc = _get_nc()
    f = lambda a: np.ascontiguousarray(a, dtype=np.float32)
    shared = host_transform(w_gate, conv_w, conv_b, bn_w, bn_b, rmean, rvar)
    in_maps = []
    for i in range(NCORES):
        sl = slice(i * BPC, (i + 1) * BPC)
        in_maps.append({"x": f(x[sl]), "gumbel": f(gumbel_noise[sl]), **shared})
    res = run_bass_kernel_spmd(nc, in_maps, list(range(NCORES)))
    out = np.concatenate([res.results[i]["out"] for i in range(NCORES)], axis=0)
    return out.astype(np.float32, copy=False)


# revision 23
# speedup vs baseline: 1.0591x; 1.0591x over previous
"""Trainium2 Bass kernel for the slimmable-conv MoE-routing module.

Reference computation (B=16, C=128, L=32768, G=4):
  pool   = mean(x, axis=-1)                      [B, C]
  logits = pool @ w_gate.T                       [B, G]
  gate   = straight-through gumbel softmax       [B, G]  (~one-hot)
  z      = conv_w @ x + conv_b                   [B, C, L]  (pointwise conv)
  out1   = z * (gate @ MASK)                     (channel gating)
  xn     = (out1 - gate@rmean) / sqrt(gate@rvar + eps) * bn_w + bn_b
  out    = xn * (gate @ MASK)

Everything after the pool reduces to a per-(batch,channel) affine applied to
the conv output:  out[b,c,l] = z_mm[b,c,l] * S[b,c] + T[b,c]  where z_mm is
the pure matmul part and S/T fold the gate, conv bias and BN constants.
The per-gate-choice tables S_all/T_all [G, C], the transposed gate weight
and the bf16 transposed conv weight are all tiny and input-independent of x,
so they are precomputed on the host in kernel() and passed as inputs.

Sharding: data-parallel over batch, 2 batches per core, 8 cores.  HBM
traffic per core is the 64 MiB floor (read x once, write out once): ALL of
z stays resident in SBUF as bf16 (0.2% rounding ~ well under the 2e-2
tolerance).  Peak residency is one batch + lookahead, not two: batch 1's z
chunks are written into the slots that batch 0's epilogue frees as its
outputs stream out.

Per-core schedule (16 chunks of 2048 cols per batch):
  phase A0 : stream b0 chunks: DMA in (Sync ring; consts ride the ACT
             ring), one DVE op casts to bf16 AND accumulates the pool
             partial-sum, PE bf16 z=conv_w@x, ACT drains PSUM -> bf16 slot.
  prefetch : b1 chunks 0..SPARE-1 -> spare slots (keeps DMA busy while
             the gate chain runs).
  gate(0)  : pool -> logits(+gumbel via PSUM accum) -> hard one-hot ->
             select S/T column via tiny matmuls.
  steady   : epilogue b0 chunk j (z*S+T -> fp32 staging -> DMA out,
             DVE/ACT alternating) interleaved with phase-A work for b1
             chunk j+SPARE into the just-freed slot.
  drain b0, gate(1), epilogue all b1 chunks (write-only tail).
"""

import ml_dtypes
import numpy as np

import concourse.bass as bass
import concourse.tile as tile
from concourse import mybir, bacc
from concourse.bass_utils import run_bass_kernel_spmd

F32 = mybir.dt.float32
BF16 = mybir.dt.bfloat16

B, C, L, G = 16, 128, 32768, 4
NCORES = 8
BPC = B // NCORES          # batches per core
CHANNELS = [32, 64, 96, 128]
EPS = 1e-5

LC = 2048                  # columns per DMA chunk
NCHUNK = L // LC           # chunks per batch
MMN = 512                  # matmul moving-dim
NMM = LC // MMN            # matmuls per chunk
SPARE = 9                  # extra z slots (b1 lookahead past gate(0))

AX = mybir.AxisListType.X
ALU = mybir.AluOpType
ACTF = mybir.ActivationFunctionType


def host_transform(w_gate, conv_w, conv_b, bn_w, bn_b, rmean, rvar,
                   l_total=L):
    """Input-side constant folding (exact fp32, matches the on-chip algebra
    that was validated against the reference).  wgT absorbs the pool's 1/L
    (L is a power of two, so the fold is exact)."""
    f = np.float32
    mask = (np.arange(C)[None, :] < np.asarray(CHANNELS)[:, None]).astype(f)
    istd = (f(1.0) / np.sqrt(np.asarray(rvar, f) + f(EPS))).astype(f)
    bw = np.asarray(bn_w, f).reshape(1, C)
    bb = np.asarray(bn_b, f).reshape(1, C)
    cb = np.asarray(conv_b, f).reshape(1, C)
    S = (mask * istd * bw).astype(f)                               # [G, C]
    T = (((cb * mask - np.asarray(rmean, f)) * istd * bw + bb) * mask).astype(f)
    return {
        "wgT": np.ascontiguousarray(
            np.asarray(w_gate, f).T * f(1.0 / l_total)),           # [C, G]
        "cwT": np.ascontiguousarray(
            np.asarray(conv_w, f).T).astype(ml_dtypes.bfloat16),   # [C, C]
        "sall": np.ascontiguousarray(S),
        "tall": np.ascontiguousarray(T),
    }


def build_kernel(l_total=L, n_res=None):
    nchunk = l_total // LC
    spare = min(SPARE, nchunk)
    nslot = nchunk + spare
    nc = bacc.Bacc("TRN2", target_bir_lowering=False)

    x_d = nc.declare_dram_parameter("x", [BPC, C, l_total], F32, isOutput=False)
    gum_d = nc.declare_dram_parameter("gumbel", [BPC, G], F32, isOutput=False)
    wg_d = nc.declare_dram_parameter("wgT", [C, G], F32, isOutput=False)
    cw_d = nc.declare_dram_parameter("cwT", [C, C], BF16, isOutput=False)
    sa_d = nc.declare_dram_parameter("sall", [G, C], F32, isOutput=False)
    ta_d = nc.declare_dram_parameter("tall", [G, C], F32, isOutput=False)
    out_d = nc.declare_dram_parameter("out", [BPC, C, l_total], F32, isOutput=True)

    def slot(b, ci):
        return ci if b == 0 else (nchunk + ci) % nslot

    with tile.TileContext(nc) as tc:
        with (
            tc.tile_pool(name="consts", bufs=1) as consts,
            tc.tile_pool(name="xin", bufs=6) as xin_pool,
            tc.tile_pool(name="xbf", bufs=3) as xbf_pool,
            tc.tile_pool(name="zres", bufs=1) as zres_pool,
            tc.tile_pool(name="stage", bufs=5) as stage_pool,
            tc.tile_pool(name="small", bufs=1) as small,
            tc.tile_pool(name="psz", bufs=5, space="PSUM") as psz,
            tc.tile_pool(name="pss", bufs=1, space="PSUM") as pss,
        ):
            # ---- prefetch the first x chunks; all consts DMAs ride the
            # ACT HWDGE ring so the Sync ring is a pure x/out stream ----
            npre = min(6, nchunk)
            pre = []
            for ci in range(npre):
                xc = xin_pool.tile([C, LC], F32, tag="xin", name=f"xpre{ci}")
                nc.sync.dma_start(
                    out=xc, in_=x_d.ap()[0, :, ci * LC:(ci + 1) * LC])
                pre.append(xc)

            # ---- constants (DMA-only; no on-chip compute) ----
            convwT = consts.tile([C, C], BF16)      # [i, o] = conv_w[o, i]
            nc.scalar.dma_start(out=convwT, in_=cw_d.ap())
            wgT = consts.tile([C, G], F32)          # [c, g] = w_gate[g, c]
            nc.scalar.dma_start(out=wgT, in_=wg_d.ap())
            S_allT = consts.tile([G, C], F32)
            nc.scalar.dma_start(out=S_allT, in_=sa_d.ap())
            T_allT = consts.tile([G, C], F32)
            nc.scalar.dma_start(out=T_allT, in_=ta_d.ap())
            gum_rows = []
            for b in range(BPC):
                gr = consts.tile([1, G], F32, tag=f"gum{b}")
                nc.scalar.dma_start(out=gr, in_=gum_d.ap()[b:b + 1, :])
                gum_rows.append(gr)

            one_sb = consts.tile([1, 1], F32)
            nc.vector.memset(one_sb, 1.0)
            partials = consts.tile([C, BPC * nchunk], F32)
            nc.vector.memset(partials, 0.0)
            pool_sb = consts.tile([C, BPC], F32)
            ST_sb = consts.tile([C, 2 * BPC], F32)

            zres = [zres_pool.tile([C, LC], BF16, tag=f"z{s}", name=f"zres{s}")
                    for s in range(nslot)]

            def phase_a_chunk(b, ci, xc=None, copy_eng="act"):
                col = b * nchunk + ci
                if xc is None:
                    xc = xin_pool.tile([C, LC], F32, tag="xin")
                    nc.sync.dma_start(
                        out=xc, in_=x_d.ap()[b, :, ci * LC:(ci + 1) * LC])
                # one DVE op: bf16 cast for the matmul + fp32 column-sum
                # (pool noise ~1e-5 logit shift vs 0.04 min gate gap)
                xbf = xbf_pool.tile([C, LC], BF16, tag="xbf")
                nc.vector.tensor_scalar(
                    out=xbf, in0=xc, scalar1=1.0, scalar2=None, op0=ALU.mult,
                    op1=ALU.add, accum_out=partials[:, col:col + 1])
                dst = zres[slot(b, ci)]
                for j in range(NMM):
                    js = slice(j * MMN, (j + 1) * MMN)
                    zp = psz.tile([C, MMN], F32)
                    nc.tensor.matmul(out=zp, lhsT=convwT, rhs=xbf[:, js],
                                     start=True, stop=True)
                    # during steady the PSUM drain must stay off ACT: ACT's
                    # FIFO is write-paced (affines wait on stage bufs) and
                    # would back-pressure PE -> xbf -> accums -> gate(1)
                    if copy_eng == "act":
                        nc.scalar.copy(out=dst[:, js], in_=zp)
                    else:
                        nc.vector.tensor_copy(out=dst[:, js], in_=zp)

            def finish_pool(b):
                # pool_sb holds column SUMS; the 1/L lives in wgT (host)
                nc.vector.reduce_sum(
                    out=pool_sb[:, b:b + 1],
                    in_=partials[:, b * nchunk:(b + 1) * nchunk],
                    axis=AX)

            def gate_phase(b):
                """Short gating chain: logits -> hard one-hot -> select
                precomputed S/T columns via tiny matmuls.  Kept as few
                serial cross-engine hops as possible (each costs ~1.5us)."""
                # y = pool @ wgT + 1*gumbel, accumulated in PSUM (2 PE ops)
                lg_ps = pss.tile([1, G], F32, tag="lg")
                nc.tensor.matmul(out=lg_ps, lhsT=pool_sb[:, b:b + 1], rhs=wgT,
                                 start=True, stop=False)
                nc.tensor.matmul(out=lg_ps, lhsT=one_sb, rhs=gum_rows[b],
                                 start=False, stop=True)
                m1 = small.tile([1, 1], F32, tag=f"m1{b}")
                nc.vector.reduce_max(out=m1, in_=lg_ps, axis=AX)
                yhard = small.tile([1, G], F32, tag=f"yh{b}")
                nc.vector.tensor_scalar(out=yhard, in0=lg_ps, scalar1=m1,
                                        scalar2=None, op0=ALU.is_ge)
                gt_ps = pss.tile([G, 1], F32, tag="gt")
                nc.tensor.transpose(out=gt_ps, in_=yhard, identity=one_sb)
                gateT = small.tile([G, 1], F32, tag=f"gT{b}")
                nc.vector.tensor_copy(out=gateT, in_=gt_ps)

                sel_ps = pss.tile([C, 2], F32, tag="big")
                nc.tensor.matmul(out=sel_ps[:, 0:1], lhsT=S_allT, rhs=gateT,
                                 start=True, stop=True)
                nc.tensor.matmul(out=sel_ps[:, 1:2], lhsT=T_allT, rhs=gateT,
                                 start=True, stop=True)
                nc.vector.tensor_copy(out=ST_sb[:, 2 * b:2 * b + 2],
                                      in_=sel_ps)

            epi_count = [0]

            def epilogue(b, ci, eng=None):
                """out[:, chunk] = zres * S + T   (bf16 -> fp32 staging)"""
                S_col = ST_sb[:, 2 * b:2 * b + 1]
                T_col = ST_sb[:, 2 * b + 1:2 * b + 2]
                zt = zres[slot(b, ci)]
                st = stage_pool.tile([C, LC], F32, tag="stage")
                use_vec = (epi_count[0] % 2 == 0) if eng is None else (eng == "vec")
                if use_vec:
                    nc.vector.tensor_scalar(
                        out=st, in0=zt, scalar1=S_col, scalar2=T_col,
                        op0=ALU.mult, op1=ALU.add)
                else:
                    nc.scalar.activation(out=st, in_=zt, func=ACTF.Identity,
                                         bias=T_col, scale=S_col)
                epi_count[0] += 1
                # out-DMAs ride the ACT HWDGE ring: on the Sync ring their
                # wait-for-affine would head-of-line block the in-DMA
                # dispatches (the read stream) behind the write pace
                nc.scalar.dma_start(
                    out=out_d.ap()[b, :, ci * LC:(ci + 1) * LC], in_=st)

            # ---- emission order ----
            # Mixed read+write DMA sustains ~400 GB/s vs ~330 write-only,
            # so steady interleaves TWO b0 epilogues per b1 fill: all of
            # b0's writes overlap b1's remaining reads.  gate(1) is emitted
            # straight after the last fill so its chain isn't queued behind
            # write-paced epilogues; the pure-write tail is then just b1.
            with nc.named_scope("phaseA0"):
                for ci in range(nchunk):
                    phase_a_chunk(0, ci, xc=pre[ci] if ci < npre else None)
            with nc.named_scope("prefetchB1"):
                for ci in range(spare):
                    phase_a_chunk(1, ci)
            with nc.named_scope("gate0"):
                finish_pool(0)
                gate_phase(0)
            epi_next = 0
            with nc.named_scope("steady"):
                for k, ci in enumerate(range(spare, nchunk)):
                    target = min(nchunk, max(ci - spare + 1, 2 * (k + 1)))
                    while epi_next < target:
                        epilogue(0, epi_next, eng="act")
                        epi_next += 1
                    phase_a_chunk(1, ci, copy_eng="vec")
            with nc.named_scope("gate1"):
                finish_pool(1)
                gate_phase(1)
            with nc.named_scope("drain"):
                while epi_next < nchunk:
                    epilogue(0, epi_next)
                    epi_next += 1
                for ci in range(nchunk):
                    epilogue(1, ci)

    nc.compile()
    return nc


_NC = None


def _get_nc():
    global _NC
    if _NC is None:
        _NC = build_kernel()
    return _NC


def kernel(x, gumbel_noise, w_gate, conv_w, conv_b, bn_w, bn_b, rmean, rvar):
    nc = _get_nc()
    f = lambda a: np.ascontiguousarray(a, dtype=np.float32)
    shared = host_transform(w_gate, conv_w, conv_b, bn_w, bn_b, rmean, rvar)
    in_maps = []
    for i in range(NCORES):
        sl = slice(i * BPC, (i + 1) * BPC)
        in_maps.append({"x": f(x[sl]), "gumbel": f(gumbel_noise[sl]), **shared})
    res = run_bass_kernel_spmd(nc, in_maps, list(range(NCORES)))
    out = np.concatenate([res.results[i]["out"] for i in range(NCORES)], axis=0)
    return out.astype(np.float32, copy=False)
